# revision 6
# baseline (speedup 1.0000x reference)
"""CrossModalCenterLoss Trainium2 kernel (Bass, raw engine programming).

Math
----
The reference builds the full [B, C] squared-distance matrix
    distmat[b, c] = ||x_b||^2 + ||center_c||^2 - 2 x_b . center_c,
multiplies by a one-hot label mask, clamps EVERY entry to [1e-12, 1e12]
(so each masked-out zero becomes exactly 1e-12), sums, and divides by B.
Equivalently:

    loss = ( sum_b clip(||x_b - centers[labels_b]||^2, 1e-12, 1e12)
             + (B*C - B) * 1e-12 ) / B

Only the B labeled center rows are ever needed, so instead of streaming the
full 51 MB centers table we gather exactly those rows with an indirect
(SWDGE) DMA: O(B*D) memory traffic instead of O(C*D).

Sharding
--------
Data-parallel over batch: 8 cores x 256 rows, centers replicated in each
core's HBM (only 256 rows of it are read per core).  Each core reduces its
256 row distances to a [1, 2] partial; the host sums the 16 partials
(the scalar "all-reduce"), adds the analytic (B*C - B)*1e-12 clamp constant
and divides by the global batch.

The per-row clamp itself is dropped on-device: for randn-distributed x and
centers every row distance sits in ~[250, 900], six-plus orders of magnitude
inside [1e-12, 1e12]; even if the lower clamp did bind somewhere, omitting
it perturbs the loss by at most B*1e-12 ~ 2e-9 absolute (~4e-12 relative).

Per-core schedule (engines run concurrently; ~6.8 us critical path):
  SP  : labels DMA -> x DMA -> (wait result) out DMA
  Pool: (wait labels) indirect gather x2 -> (wait rows) partition-reduce
  DVE : diff0 -> diff1 -> square+rowsum tile1
  ACT : warm-up (loads Square table under the DMAs) -> square+rowsum tile0
"""

import numpy as np

B = 2048
D = 256
C = 50000
NCORES = 8
P = 128
BS = B // NCORES  # 256 rows per core; SBUF row (p, t) holds shard row 2p+t
CLAMP_MIN = 1e-12
CLAMP_MAX = 1e12

_CACHE = {}


def _build_nc():
    import concourse.bass as bass
    import concourse.mybir as mybir

    f32 = mybir.dt.float32
    i32 = mybir.dt.int32

    nc = bass.Bass("TRN2")
    x = nc.dram_tensor("x", [BS, D], f32, kind="ExternalInput")
    labels = nc.dram_tensor("labels", [BS, 1], i32, kind="ExternalInput")
    centers = nc.dram_tensor("centers", [C, D], f32, kind="ExternalInput")
    out = nc.dram_tensor("out", [1, 2], f32, kind="ExternalOutput")

    with (
        nc.sbuf_tensor([P, 2], i32) as lab,
        nc.sbuf_tensor([P, 2 * D], f32) as xt,
        nc.sbuf_tensor([P, 2 * D], f32) as ct,
        nc.sbuf_tensor([P, 2 * D], f32) as diff,
        nc.sbuf_tensor([P, 2 * D], f32) as sq,
        nc.sbuf_tensor([P, 2], f32) as row,
        nc.sbuf_tensor([1, 2], f32) as res,
        nc.sbuf_tensor([1, 1], f32) as warm,
        nc.semaphore() as lab_sem,
        nc.semaphore() as x_sem,
        nc.semaphore() as g0_sem,
        nc.semaphore() as g1_sem,
        nc.semaphore() as out_sem,
        nc.semaphore() as s_dve,
        nc.semaphore() as s_act,
        nc.semaphore() as s_pool,
        nc.semaphore() as s_warm,
        nc.Block() as block,
    ):
        sl0 = slice(0, D)
        sl1 = slice(D, 2 * D)

        @block.sync
        def _(sync):
            sync.dma_start(
                out=lab[:].rearrange("p (t o) -> p t o", o=1),
                in_=labels.rearrange("(p t) o -> p t o", t=2),
            ).then_inc(lab_sem, 16)
            sync.dma_start(
                out=xt[:].rearrange("p (t d) -> p t d", d=D),
                in_=x.rearrange("(p t) d -> p t d", t=2),
            ).then_inc(x_sem, 16)
            sync.wait_ge(s_pool, 1)
            sync.dma_start(out=out[:, :], in_=res[:]).then_inc(out_sem, 16)
            sync.wait_ge(out_sem, 16)

        @block.gpsimd
        def _(gpsimd):
            gpsimd.wait_ge(lab_sem, 16)
            for t, g_sem in ((0, g0_sem), (1, g1_sem)):
                gpsimd.indirect_dma_start(
                    out=ct[:, t * D : (t + 1) * D],
                    out_offset=None,
                    in_=centers[:],
                    in_offset=bass.IndirectOffsetOnAxis(
                        ap=lab[:, t : t + 1], axis=0
                    ),
                ).then_inc(g_sem, 16)
            gpsimd.wait_ge(s_dve, 3)
            gpsimd.wait_ge(s_act, 1)
            gpsimd.tensor_reduce(
                out=res[:],
                in_=row[:],
                axis=mybir.AxisListType.C,
                op=mybir.AluOpType.add,
            ).then_inc(s_pool, 1)

        @block.vector
        def _(vector):
            vector.memset(warm[:], 0.0).then_inc(s_warm, 1)
            vector.wait_ge(x_sem, 16)
            # diff0 -> ACT squares it; diff1 -> DVE squares it (fused w/accum)
            vector.wait_ge(g0_sem, 16)
            vector.tensor_sub(diff[:, sl0], xt[:, sl0], ct[:, sl0]).then_inc(
                s_dve, 1
            )
            vector.wait_ge(g1_sem, 16)
            vector.tensor_sub(diff[:, sl1], xt[:, sl1], ct[:, sl1]).then_inc(
                s_dve, 1
            )
            vector.wait_ge(s_dve, 2)
            vector.scalar_tensor_tensor(
                out=sq[:, sl1],
                in0=diff[:, sl1],
                scalar=0.0,
                in1=diff[:, sl1],
                op0=mybir.AluOpType.add,
                op1=mybir.AluOpType.mult,
                accum_out=row[:, 1:2],
            ).then_inc(s_dve, 1)

        @block.scalar
        def _(scalar):
            # Warm-up loads the Square piecewise-poly table while the input
            # DMAs run, instead of serializing it into the first activation.
            scalar.wait_ge(s_warm, 1)
            scalar.activation(
                out=warm[:],
                in_=warm[:],
                func=mybir.ActivationFunctionType.Square,
            )
            scalar.wait_ge(s_dve, 1)
            scalar.activation(
                out=sq[:, sl0],
                in_=diff[:, sl0],
                func=mybir.ActivationFunctionType.Square,
                accum_out=row[:, 0:1],
            ).then_inc(s_act, 1)

    nc.finalize()
    return nc


def kernel(x, labels, centers):
    if "nc" not in _CACHE:
        _CACHE["nc"] = _build_nc()
    nc = _CACHE["nc"]
    from concourse.bass_utils import run_bass_kernel_spmd

    x = np.ascontiguousarray(np.asarray(x, dtype=np.float32).reshape(B, D))
    labels_i32 = np.ascontiguousarray(
        np.asarray(labels).astype(np.int32).reshape(B, 1)
    )
    centers = np.ascontiguousarray(np.asarray(centers, dtype=np.float32))

    in_maps = [
        {
            "x": np.ascontiguousarray(x[c * BS : (c + 1) * BS]),
            "labels": np.ascontiguousarray(labels_i32[c * BS : (c + 1) * BS]),
            "centers": centers,
        }
        for c in range(NCORES)
    ]
    res = run_bass_kernel_spmd(nc, in_maps, core_ids=list(range(NCORES)))
    total = float(
        np.sum(
            np.stack([r["out"] for r in res.results]).astype(np.float64)
        )
    )
    total += (B * C - B) * CLAMP_MIN  # every masked-out entry clamps to 1e-12
    return np.array(total / B, dtype=np.float32)


# revision 7
# speedup vs baseline: 1.0105x; 1.0105x over previous
"""CrossModalCenterLoss Trainium2 kernel (Bass, raw engine programming).

Math
----
The reference builds the full [B, C] squared-distance matrix
    distmat[b, c] = ||x_b||^2 + ||center_c||^2 - 2 x_b . center_c,
multiplies by a one-hot label mask, clamps EVERY entry to [1e-12, 1e12]
(so each masked-out zero becomes exactly 1e-12), sums, and divides by B.
Equivalently:

    loss = ( sum_b clip(||x_b - centers[labels_b]||^2, 1e-12, 1e12)
             + (B*C - B) * 1e-12 ) / B

Only the B labeled center rows are ever needed, so instead of streaming the
full 51 MB centers table we gather exactly those rows with an indirect
(SWDGE) DMA: O(B*D) memory traffic instead of O(C*D).

Sharding
--------
Data-parallel over batch: 8 cores x 256 rows, centers replicated in each
core's HBM (only 256 rows of it are read per core).  Each core writes its
[128, 2] per-row squared distances; the host sums the 8 partials (the
all-reduce/unshard step), adds the analytic (B*C - B)*1e-12 clamp constant
and divides by the global batch.

The per-row clamp itself is dropped on-device: for randn-distributed x and
centers every row distance sits in ~[250, 900], six-plus orders of magnitude
inside [1e-12, 1e12]; even if the lower clamp did bind somewhere, omitting
it perturbs the loss by at most B*1e-12 ~ 2e-9 absolute (~4e-12 relative).

Per-core schedule (engines run concurrently; ~6.7 us critical path):
  SP  : labels DMA -> x DMA -> (wait rows) out DMA
  Pool: (wait labels) indirect gather x2
  DVE : diff0 -> diff1 -> square+rowsum tile1
  ACT : warm-up (loads Square table under the DMAs) -> square+rowsum tile0
"""

import numpy as np

B = 2048
D = 256
C = 50000
NCORES = 8
P = 128
BS = B // NCORES  # 256 rows per core; SBUF row (p, t) holds shard row 2p+t
CLAMP_MIN = 1e-12
CLAMP_MAX = 1e12

_CACHE = {}


def _build_nc():
    import concourse.bass as bass
    import concourse.mybir as mybir

    f32 = mybir.dt.float32
    i32 = mybir.dt.int32

    nc = bass.Bass("TRN2")
    x = nc.dram_tensor("x", [BS, D], f32, kind="ExternalInput")
    labels = nc.dram_tensor("labels", [BS, 1], i32, kind="ExternalInput")
    centers = nc.dram_tensor("centers", [C, D], f32, kind="ExternalInput")
    out = nc.dram_tensor("out", [P, 2], f32, kind="ExternalOutput")

    with (
        nc.sbuf_tensor([P, 2], i32) as lab,
        nc.sbuf_tensor([P, 2 * D], f32) as xt,
        nc.sbuf_tensor([P, 2 * D], f32) as ct,
        nc.sbuf_tensor([P, 2 * D], f32) as diff,
        nc.sbuf_tensor([P, 2 * D], f32) as sq,
        nc.sbuf_tensor([P, 2], f32) as row,
        nc.sbuf_tensor([1, 1], f32) as warm,
        nc.semaphore() as lab_sem,
        nc.semaphore() as x_sem,
        nc.semaphore() as g0_sem,
        nc.semaphore() as g1_sem,
        nc.semaphore() as out_sem,
        nc.semaphore() as s_dve,
        nc.semaphore() as s_act,
        nc.semaphore() as s_warm,
        nc.Block() as block,
    ):
        sl0 = slice(0, D)
        sl1 = slice(D, 2 * D)

        @block.sync
        def _(sync):
            sync.dma_start(
                out=lab[:].rearrange("p (t o) -> p t o", o=1),
                in_=labels.rearrange("(p t) o -> p t o", t=2),
            ).then_inc(lab_sem, 16)
            sync.dma_start(
                out=xt[:].rearrange("p (t d) -> p t d", d=D),
                in_=x.rearrange("(p t) d -> p t d", t=2),
            ).then_inc(x_sem, 16)
            sync.wait_ge(s_act, 1)
            sync.wait_ge(s_dve, 3)
            sync.dma_start(out=out[:, :], in_=row[:]).then_inc(out_sem, 16)
            sync.wait_ge(out_sem, 16)

        @block.gpsimd
        def _(gpsimd):
            gpsimd.wait_ge(lab_sem, 16)
            for t, g_sem in ((0, g0_sem), (1, g1_sem)):
                gpsimd.indirect_dma_start(
                    out=ct[:, t * D : (t + 1) * D],
                    out_offset=None,
                    in_=centers[:],
                    in_offset=bass.IndirectOffsetOnAxis(
                        ap=lab[:, t : t + 1], axis=0
                    ),
                ).then_inc(g_sem, 16)

        @block.vector
        def _(vector):
            vector.memset(warm[:], 0.0).then_inc(s_warm, 1)
            vector.wait_ge(x_sem, 16)
            # diff0 -> ACT squares it; diff1 -> DVE squares it (fused w/accum)
            vector.wait_ge(g0_sem, 16)
            vector.tensor_sub(diff[:, sl0], xt[:, sl0], ct[:, sl0]).then_inc(
                s_dve, 1
            )
            vector.wait_ge(g1_sem, 16)
            vector.tensor_sub(diff[:, sl1], xt[:, sl1], ct[:, sl1]).then_inc(
                s_dve, 1
            )
            vector.wait_ge(s_dve, 2)
            vector.scalar_tensor_tensor(
                out=sq[:, sl1],
                in0=diff[:, sl1],
                scalar=0.0,
                in1=diff[:, sl1],
                op0=mybir.AluOpType.add,
                op1=mybir.AluOpType.mult,
                accum_out=row[:, 1:2],
            ).then_inc(s_dve, 1)

        @block.scalar
        def _(scalar):
            # Warm-up loads the Square piecewise-poly table while the input
            # DMAs run, instead of serializing it into the first activation.
            scalar.wait_ge(s_warm, 1)
            scalar.activation(
                out=warm[:],
                in_=warm[:],
                func=mybir.ActivationFunctionType.Square,
            )
            scalar.wait_ge(s_dve, 1)
            scalar.activation(
                out=sq[:, sl0],
                in_=diff[:, sl0],
                func=mybir.ActivationFunctionType.Square,
                accum_out=row[:, 0:1],
            ).then_inc(s_act, 1)

    nc.finalize()
    return nc


def kernel(x, labels, centers):
    if "nc" not in _CACHE:
        _CACHE["nc"] = _build_nc()
    nc = _CACHE["nc"]
    from concourse.bass_utils import run_bass_kernel_spmd

    x = np.ascontiguousarray(np.asarray(x, dtype=np.float32).reshape(B, D))
    labels_i32 = np.ascontiguousarray(
        np.asarray(labels).astype(np.int32).reshape(B, 1)
    )
    centers = np.ascontiguousarray(np.asarray(centers, dtype=np.float32))

    in_maps = [
        {
            "x": np.ascontiguousarray(x[c * BS : (c + 1) * BS]),
            "labels": np.ascontiguousarray(labels_i32[c * BS : (c + 1) * BS]),
            "centers": centers,
        }
        for c in range(NCORES)
    ]
    res = run_bass_kernel_spmd(nc, in_maps, core_ids=list(range(NCORES)))
    # Unshard: the [128, 2] per-core row partials concatenate to the 2048
    # per-sample distances; the final sum is the cross-core all-reduce.
    total = float(
        np.sum(
            np.stack([r["out"] for r in res.results]).astype(np.float64)
        )
    )
    total += (B * C - B) * CLAMP_MIN  # every masked-out entry clamps to 1e-12
    return np.array(total / B, dtype=np.float32)
